# revision 1
# baseline (speedup 1.0000x reference)
"""AdaptiveCLPL loss on 8 TRN2 NeuronCores (Bass/Tile).

loss = mean_b [ psi(avg_cand) + sum_head psi(-l)*(1-mask) + ts*sum_samp psi(-l)*(1-is_cand) ]
with psi(u) = softplus(-u), so psi(-l) = softplus(l) = Ln(Exp(l)+1) (composite,
the act tables here don't expose native Softplus).

Decomposition (only term1 is per-row nonlinear; everything else sums):
  total = sum_b softplus(-avg_b)
        + [sum_{head block} softplus(l)    - sum_k uniq*inhead*softplus(l_cand)]
        + ts*[sum_{sampled rows} softplus(l) - sum_k uniq*mult*softplus(l_cand)]
  uniq/mult/inhead are pure index functions -> computed on HOST; all logit
  VALUES are read and combined on device.

Per-core layout: transposed batch shard lT = logits[rows].T ([C, RB] row-major).
- head block rows [0, HEAD): one DMA with 16KB/partition descriptors +
  fused Exp/Ln(+1) with accum row-sums.
- sampled rows: one indirect row-gather (100 x 1KB descriptors).
- candidate logits: viewed as a [2C, 128] chunk table; <=4 dma_gather calls
  (int16 index windows of 32768 chunks) pull one 512B chunk per candidate in
  a single instruction each; a host-built one-hot mask extracts the element.
Slot layout: candidate (b,k) -> partition b%128; per-(bucket,partition) column
lists padded to the bucket's max column count (shared across cores).
"""

import os
import numpy as np

B, C, K = 2048, 50000, 10
HEAD, S = 2000, 100
TSCALE = float(C - HEAD) / float(S)  # 480.0
NCORES = 8
RB = B // NCORES  # 256 rows per core
P = 128
HP = 125          # head tile partitions; 2000 rows = 125 * 16
HB = HEAD // HP   # 16 blocks of 256 -> 16KB contiguous per partition
ES = 256          # dma_gather chunk = one lT row (1KB)
CHUNKS = C * RB // ES           # 100000 chunks in the flat shard
WIN = 32768                     # int16 index window
NBUCKETS = (CHUNKS + WIN - 1) // WIN  # 4
GMAX = 4                        # max rows packed per partition

_CACHE = {}


def prep_inputs(logits, candidates, sampled_indices):
    """Full inputs -> (in_maps, meta). Host work is sharding + index math only."""
    logits = np.asarray(logits)
    candidates = np.asarray(candidates)
    sampled_indices = np.asarray(sampled_indices)
    assert logits.shape == (B, C) and candidates.shape == (B, K)
    srow = (HEAD + sampled_indices.astype(np.int64)).astype(np.int32)
    sidx = srow.reshape(S, 1)
    # multiplicity of each sampled column value
    svals, scounts = np.unique(srow, return_counts=True)
    smult = dict(zip(svals.tolist(), scounts.tolist()))

    cores = []
    for i in range(NCORES):
        rows = slice(i * RB, (i + 1) * RB)
        cand = candidates[rows].astype(np.int64)          # [RB, K]
        valid = cand >= 0
        # uniq: first occurrence within the row (k order)
        uniq = valid.copy()
        for k in range(1, K):
            dup = (cand[:, :k] == cand[:, k:k + 1]).any(axis=1)
            uniq[:, k] &= ~dup
        uniqf = uniq.astype(np.float32)
        cnt = np.maximum(uniq.sum(axis=1), 1).astype(np.float32)   # [RB]
        inhead = (cand < HEAD).astype(np.float32)
        mult = np.vectorize(lambda c: smult.get(int(c), 0))(cand).astype(np.float32)
        wcorr_rk = -uniqf * (inhead + TSCALE * mult)               # [RB, K]
        flat = cand * RB + np.arange(RB)[:, None]                  # [RB, K]
        chunk, off = flat // ES, flat % ES
        wbkt, idx_local = chunk // WIN, chunk % WIN

        # balance: assign rows -> (partition, group<GMAX) greedily by
        # per-bucket loads; dummy (p,g) slots are masked via abias.
        rowcnt = np.zeros((RB, NBUCKETS), np.int64)
        for w in range(NBUCKETS):
            rowcnt[:, w] = (wbkt == w).sum(axis=1)
        base_order = np.argsort(-rowcnt.max(axis=1), kind="stable")
        lb = np.maximum(
            np.ceil(rowcnt.sum(axis=0) / P), rowcnt.max(axis=0)).astype(
                np.int64)
        rng_pack = np.random.default_rng(12345)

        def pack(order, caps):
            load = np.zeros((P, NBUCKETS), np.int64)
            slots_left = np.full(P, GMAX)
            rowpart = np.zeros(RB, np.int64)
            for r in order.tolist():
                fits = np.where(
                    (slots_left > 0)
                    & ((load + rowcnt[r]) <= caps).all(axis=1))[0]
                if len(fits) == 0:
                    return None
                p = fits[np.argmax((load[fits] * rowcnt[r]).sum(axis=1)
                                   + (GMAX - slots_left[fits]))]
                rowpart[r] = p
                load[p] += rowcnt[r]
                slots_left[p] -= 1
            return rowpart, load

        orders = [base_order,
                  np.argsort(-rowcnt[:, :1].sum(axis=1), kind="stable")]
        orders += [rng_pack.permutation(RB) for _ in range(30)]
        best = None
        for extra in range(0, 64):
            # distribute the slack `extra` over buckets, tightest first
            caps = lb.copy()
            rem = extra
            for w in np.argsort(lb)[::-1]:
                add = min(rem, 2)
                caps[w] += add
                rem -= add
            if rem:
                caps += int(np.ceil(rem / NBUCKETS))
            for order in orders:
                got = pack(order, caps)
                if got is not None:
                    score = got[1].max(axis=0).sum()
                    if best is None or score < best[0]:
                        best = (score, got[0].copy(), got[1].copy())
            if best is not None:
                break
        assert best is not None, "row packing failed"
        _, rowpart, load = best
        # group index = arrival order within the partition
        rowgrp = np.zeros(RB, np.int64)
        seen_per_p = {}
        for r in range(RB):
            p = int(rowpart[r])
            g = seen_per_p.get(p, 0)
            seen_per_p[p] = g + 1
            rowgrp[r] = g
        assert max(seen_per_p.values()) <= GMAX
        cores.append((cand, uniqf, cnt, wcorr_rk, idx_local, off, wbkt,
                      rowpart, rowgrp, load))

    nj = [0] * NBUCKETS
    for core in cores:
        load = core[-1]
        for w in range(NBUCKETS):
            nj[w] = max(nj[w], int(load[:, w].max()))
    active = [w for w in range(NBUCKETS) if nj[w] > 0]
    njs = [nj[w] for w in active]
    njtot = sum(njs)
    j0 = {}
    acc = 0
    for w, n in zip(active, njs):
        j0[w] = acc
        acc += n
    n16 = [n * P // 16 for n in njs]

    in_maps = []
    for i, core in enumerate(cores):
        (cand, uniqf, cnt, wcorr_rk, idx_local, off, wbkt,
         rowpart, rowgrp, load) = core
        rows = slice(i * RB, (i + 1) * RB)
        lT = np.ascontiguousarray(logits[rows].T.astype(np.float32, copy=False))
        idx16 = np.zeros((P, sum(n16)), np.int16)
        offt = np.full((P, njtot), -1.0, np.float32)
        wcorr = np.zeros((P, njtot), np.float32)
        wg = np.zeros((P, GMAX * njtot), np.float32)
        fill = np.zeros((P, NBUCKETS), np.int64)
        o16 = 0
        idxs_w = {w: np.zeros(n * P, np.int16) for w, n in zip(active, njs)}
        for b in range(RB):
            p, g = int(rowpart[b]), int(rowgrp[b])
            for k in range(K):
                w = int(wbkt[b, k])
                j = int(fill[p, w])
                fill[p, w] += 1
                idxs_w[w][j * P + p] = idx_local[b, k]
                jj = j0[w] + j
                offt[p, jj] = float(off[b, k])
                wcorr[p, jj] = wcorr_rk[b, k]
                wg[p, g * njtot + jj] = uniqf[b, k]
        rcnt = np.ones((P, GMAX), np.float32)
        rcnt[rowpart, rowgrp] = 1.0 / cnt
        abias = np.full((P, GMAX), 40.0, np.float32)
        abias[rowpart, rowgrp] = 0.0
        for w, n, nn in zip(active, njs, n16):
            wrapped = idxs_w[w].reshape(n * P // 16, 16).T
            idx16[:, o16:o16 + nn] = np.tile(wrapped, (8, 1))
            o16 += nn
        iota128 = np.broadcast_to(
            np.arange(ES, dtype=np.float32), (P, ES)).copy()
        auxcat = np.ascontiguousarray(np.concatenate(
            [offt, iota128, wcorr, wg, rcnt, abias], axis=1))
        in_maps.append({
            "lT": lT,
            "sidx": sidx,
            "idx16": np.ascontiguousarray(idx16),
            "aux": auxcat,
        })
    meta = (tuple(active), tuple(njs))
    return in_maps, meta


def _build(meta, enable_asserts=False):
    import concourse.bass as bass
    import concourse.tile as tile
    from concourse import bacc, bass_isa, mybir
    from concourse.bass import _add_dep_helper

    active, njs = meta
    njtot = sum(njs)
    n16s = [nj * P // 16 for nj in njs]
    n16tot = sum(n16s)

    f32 = mybir.dt.float32
    i32 = mybir.dt.int32
    i16 = mybir.dt.int16
    AF = mybir.ActivationFunctionType
    OP = mybir.AluOpType
    AX = mybir.AxisListType

    nc = bacc.Bacc(
        "TRN2",
        target_bir_lowering=False,
        debug=False,
        enable_asserts=enable_asserts,
        num_devices=NCORES,
    )

    lT = nc.dram_tensor("lT", [C, RB], f32, kind="ExternalInput").ap()
    sidx = nc.dram_tensor("sidx", [S, 1], i32, kind="ExternalInput").ap()
    idx16 = nc.dram_tensor("idx16", [P, n16tot], i16, kind="ExternalInput").ap()
    AUXW = njtot * (2 + GMAX) + ES + 2 * GMAX
    aux = nc.dram_tensor("aux", [P, AUXW], f32, kind="ExternalInput").ap()
    out = nc.dram_tensor("out", [1, 1], f32, kind="ExternalOutput").ap()

    # chunk-table view of the shard: [2C, 128] rows of 512B
    ctab = lT.rearrange("a (b c) -> (a b) c", c=ES)

    with tile.TileContext(nc) as tc:
        with tc.tile_pool(name="sb", bufs=1) as sb:
            total = sb.tile([P, 1], f32)
            nc.vector.memset(total[:, :], 0.0)

            # ---- A: index DMAs + gathers (gpsimd work starts early) ----
            sidx_t = sb.tile([S, 1], i32)
            d_sidx = nc.gpsimd.dma_start(out=sidx_t[:, :], in_=sidx[:, :])
            idx16_t = sb.tile([P, n16tot], i16)
            d_idx16 = nc.gpsimd.dma_start(out=idx16_t[:, :], in_=idx16[:, :])

            samp = sb.tile([S, RB], f32)
            d_samp = nc.gpsimd.indirect_dma_start(
                out=samp[:, :], out_offset=None, in_=lT[:, :],
                in_offset=bass.IndirectOffsetOnAxis(ap=sidx_t[:, :1], axis=0))

            gdst = sb.tile([P, njtot * ES], f32)
            gathers = []
            o16 = 0
            jo = 0
            for w, nj, nn in zip(active, njs, n16s):
                lo = w * WIN
                hi = min(CHUNKS, lo + WIN)
                gathers.append(nc.gpsimd.dma_gather(
                    out_ap=gdst[:, jo * ES:(jo + nj) * ES].rearrange(
                        "p (j e) -> p j e", e=ES),
                    in_ap=ctab[lo:hi, :],
                    idxs_ap=idx16_t[:, o16:o16 + nn],
                    num_idxs=nj * P,
                    num_idxs_reg=nj * P,
                    elem_size=ES,
                    single_packet=False,
                ))
                o16 += nn
                jo += nj

            # ---- B: head DMA split across both HWDGE rings ----
            ht = sb.tile([HP, HB * RB], f32)
            hsrc = lT[:HEAD, :].rearrange("(p j) c -> p (j c)", j=HB)
            half = HB * RB // 2
            d_h0 = nc.sync.dma_start(out=ht[:, :half], in_=hsrc[:, :half])
            d_h1 = nc.scalar.dma_start(out=ht[:, half:], in_=hsrc[:, half:])
            for d in (d_h0, d_h1):
                # real sem wait on the sampled gather: keeps the wire empty
                # while the tiny index DMA completions gate the gather chain
                # (an in-flight 2MB HWDGE transfer delays them ~12-16us), yet
                # starts the head early enough that its Exp/Ln still finishes
                # inside the gather shadow.
                _add_dep_helper(d.ins, d_samp.ins, sync=True,
                                reason="bulk head after sampled gather")

            # single aux DMA (late-phase inputs), sliced below
            aux_t = sb.tile([P, AUXW], f32)
            nc.sync.dma_start(out=aux_t[:, :], in_=aux[:, :])
            o = 0
            offt_t = aux_t[:, o:o + njtot]; o += njtot
            iota_t = aux_t[:, o:o + ES]; o += ES
            wcorr_t = aux_t[:, o:o + njtot]; o += njtot
            wg_t = aux_t[:, o:o + GMAX * njtot]; o += GMAX * njtot
            rcnt_t = aux_t[:, o:o + GMAX]; o += GMAX
            abias_t = aux_t[:, o:o + GMAX]; o += GMAX

            # ---- C: bulk Exps then bulk Lns (2 act-table loads) ----
            e_h = nc.scalar.activation(ht[:, :], ht[:, :], AF.Exp)
            e_s = nc.scalar.activation(samp[:, :], samp[:, :], AF.Exp)
            hacc = sb.tile([HP, 1], f32)
            ln_h = nc.scalar.activation(ht[:, :], ht[:, :], AF.Ln, bias=1.0,
                                        accum_out=hacc[:, :])
            _add_dep_helper(ln_h.ins, e_s.ins, sync=False,
                            reason="bulk Exps before bulk Lns")
            sacc = sb.tile([S, 1], f32)
            ln_s = nc.scalar.activation(samp[:, :], samp[:, :], AF.Ln,
                                        bias=1.0, accum_out=sacc[:, :])
            _add_dep_helper(ln_s.ins, e_s.ins, sync=False,
                            reason="bulk Exps before bulk Lns")

            nc.vector.tensor_tensor(total[:HP, :], total[:HP, :], hacc[:, :],
                                    op=OP.add)
            sacc2 = sb.tile([S, 1], f32)
            nc.vector.tensor_scalar_mul(sacc2[:, :], sacc[:, :], TSCALE)
            tadd = nc.vector.tensor_tensor(total[:S, :], total[:S, :],
                                           sacc2[:, :], op=OP.add)

            # dummy Exp: reload the exp table during the gather window
            dummy = sb.tile([1, 1], f32)
            dex = nc.scalar.activation(dummy[:, :], total[0:1, :1], AF.Exp,
                                       scale=0.0)
            _add_dep_helper(dex.ins, tadd.ins, sync=False,
                            reason="prefetch exp table after bulk Lns")

            # ---- late phase: extract candidate values (per bucket) ----
            val = sb.tile([P, njtot], f32)
            jo2 = 0
            for w, nj in zip(active, njs):
                msk = sb.tile([P, nj * ES], f32, tag="msk", bufs=2)
                nc.vector.tensor_tensor(
                    out=msk[:, :].rearrange("p (j e) -> p j e", e=ES),
                    in0=iota_t.unsqueeze(1).to_broadcast([P, nj, ES]),
                    in1=offt_t[:, jo2:jo2 + nj].unsqueeze(2).to_broadcast(
                        [P, nj, ES]),
                    op=OP.is_equal)
                nc.vector.tensor_tensor(
                    msk[:, :], msk[:, :], gdst[:, jo2 * ES:(jo2 + nj) * ES],
                    op=OP.mult)
                nc.vector.tensor_reduce(
                    val[:, jo2:jo2 + nj],
                    msk[:, :].rearrange("p (j e) -> p j e", e=ES),
                    AX.X, OP.add)
                jo2 += nj

            ce = sb.tile([P, njtot], f32)
            e1 = nc.scalar.activation(ce[:, :], val[:, :], AF.Exp)
            _add_dep_helper(e1.ins, dex.ins, sync=False,
                            reason="late Exps after table prefetch")

            csum = sb.tile([P, GMAX], f32)
            scr2 = sb.tile([P, GMAX * njtot], f32)
            for g in range(GMAX):
                nc.vector.tensor_tensor(
                    scr2[:, g * njtot:(g + 1) * njtot],
                    wg_t[:, g * njtot:(g + 1) * njtot], val[:, :], op=OP.mult)
            nc.vector.tensor_reduce(
                csum[:, :],
                scr2[:, :].rearrange("p (g j) -> p g j", g=GMAX),
                AX.X, OP.add)
            avg = sb.tile([P, GMAX], f32)
            nc.vector.tensor_tensor(avg[:, :], csum[:, :], rcnt_t,
                                    op=OP.mult)
            nc.vector.tensor_tensor(avg[:, :], avg[:, :], abias_t,
                                    op=OP.add)
            ae = sb.tile([P, GMAX], f32)
            e2 = nc.scalar.activation(ae[:, :], avg[:, :], AF.Exp, scale=-1.0)

            spl = sb.tile([P, njtot], f32)
            l1 = nc.scalar.activation(spl[:, :], ce[:, :], AF.Ln, bias=1.0)
            _add_dep_helper(l1.ins, e2.ins, sync=False,
                            reason="late Exps before late Lns")
            t1 = sb.tile([P, GMAX], f32)
            t1col = sb.tile([P, 1], f32)
            nc.scalar.activation(t1[:, :], ae[:, :], AF.Ln, bias=1.0,
                                 accum_out=t1col[:, :])

            corr = sb.tile([P, 1], f32)
            scr3 = sb.tile([P, njtot], f32)
            nc.vector.tensor_tensor(scr3[:, :], wcorr_t, spl[:, :],
                                    op=OP.mult)
            nc.vector.tensor_reduce(corr[:, :], scr3[:, :], AX.X, OP.add)

            nc.vector.tensor_tensor(total[:, :], total[:, :], t1col[:, :],
                                    op=OP.add)
            nc.vector.tensor_tensor(total[:, :], total[:, :], corr[:, :],
                                    op=OP.add)
            gtot = sb.tile([P, 1], f32)
            nc.gpsimd.partition_all_reduce(gtot[:, :], total[:, :],
                                           channels=P,
                                           reduce_op=bass_isa.ReduceOp.add)
            res = sb.tile([1, 1], f32)
            nc.vector.tensor_scalar_mul(res[:, :], gtot[0:1, :], 1.0 / B)
            nc.sync.dma_start(out=out[:, :], in_=res[:, :])

    nc.compile()
    return nc


def get_graph(meta, enable_asserts=False):
    key = (meta, enable_asserts)
    if key not in _CACHE:
        _CACHE[key] = _build(meta, enable_asserts=enable_asserts)
    return _CACHE[key]


def run(logits, candidates, sampled_indices, trace=False, **kw):
    """Returns (scalar float32 loss, BassKernelResults)."""
    from concourse.bass_utils import run_bass_kernel_spmd

    in_maps, meta = prep_inputs(logits, candidates, sampled_indices)
    nc = get_graph(meta)
    res = run_bass_kernel_spmd(nc, in_maps, core_ids=list(range(NCORES)),
                               trace=trace, **kw)
    partials = [r["out"].reshape(()) for r in res.results]
    loss = np.float32(np.sum(np.stack(partials), dtype=np.float64))
    return loss, res


def kernel(logits, candidates, sampled_indices):
    loss, _ = run(logits, candidates, sampled_indices, trace=False)
    return loss

